# revision 2
# baseline (speedup 1.0000x reference)
"""BiLSTM-CRF loss kernel for Trainium2 (8 NeuronCores, SPMD).

Device: the two 8192-step LSTM chains (forward on core 0, backward on core 1,
same SPMD program, data-differentiated; cores 2-7 run the program on zeros and
contribute masked-out zero partials). Each chain core JIT-computes its input
projection (x @ w_ih.T) block-by-block, runs the recurrence with gates laid
out [128 partitions x 8 gate-columns], and emits its half of the emission
matrix (h_dir @ w_out_dir.T) per 128-step block.

Host: final CRF dynamic programs (forward logsumexp, Viterbi, gold score) are
sequential 17-wide recursions identical to the reference implementation.
"""
import sys
import numpy as np

sys.path.insert(0, "/opt/trn_rl_repo")

import concourse.bass as bass
import concourse.bacc as bacc
import concourse.tile as tile
import concourse.mybir as mybir
from concourse.bass_utils import run_bass_kernel_spmd

F32 = mybir.dt.float32
F32R = mybir.dt.float32r
AF = mybir.ActivationFunctionType

S, E, H, T = 8192, 1024, 512, 17
H2 = H // 2

GATE_PERM = np.concatenate([
    np.arange(0, 256),      # i
    np.arange(256, 512),    # f
    np.arange(768, 1024),   # o
    np.arange(512, 768),    # g~ (tanh gate, pre-scaled by 2 for sigmoid trick)
])


def _host_prep_dir(w_ih, w_hh, b_ih, b_hh, w_out, off):
    wih_perm = w_ih[GATE_PERM].copy()
    whh_perm = w_hh[GATE_PERM].copy()
    b_comb = (b_ih + b_hh)[GATE_PERM].copy()
    # tanh(z) = 2*sigmoid(2z) - 1: fold the 2x into the g~ gate rows so one
    # sigmoid activation covers all 8 gate columns.
    wih_perm[768:1024] *= 2.0
    whh_perm[768:1024] *= 2.0
    b_comb[768:1024] *= 2.0
    B = wih_perm.reshape(8, 128, 8, 128)                 # [j, m, ke, p]
    wih_t = np.ascontiguousarray(B.transpose(3, 2, 0, 1)).reshape(128, 8 * 8 * 128)
    A = whh_perm.reshape(8, 128, 2, 128)                 # [j, m, k, p]
    whh_t = np.ascontiguousarray(A.transpose(3, 2, 0, 1)).reshape(128, 2 * 8 * 128)
    bias = np.ascontiguousarray(b_comb.reshape(8, 128).T)
    C = w_out[:, off:off + 256].T.reshape(2, 128, 17)    # [j, p, tau]
    woT = np.ascontiguousarray(C.transpose(1, 0, 2)).reshape(128, 34)
    return (wih_t.astype(np.float32), whh_t.astype(np.float32),
            bias.astype(np.float32), woT.astype(np.float32))


def _rview(ap, shape):
    names = "abcdefg"
    src = f"{names[0]} ({' '.join(names[1:len(shape)])})"
    dst = ' '.join(names[:len(shape)])
    kw = {names[i]: shape[i] for i in range(1, len(shape))}
    return ap.rearrange(f"{src} -> {dst}", **kw)


def _build_chain_program():
    nc = bacc.Bacc("TRN2", target_bir_lowering=False, debug=False, num_devices=8)
    xT_t = nc.dram_tensor("xT", [1024, S], F32, kind="ExternalInput")
    wih_tt = nc.dram_tensor("wih_t", [128, 8 * 8 * 128], F32, kind="ExternalInput")
    whh_tt = nc.dram_tensor("whh_t", [128, 2 * 8 * 128], F32, kind="ExternalInput")
    bias_t = nc.dram_tensor("bias", [128, 8], F32, kind="ExternalInput")
    woT_t = nc.dram_tensor("woT", [128, 34], F32, kind="ExternalInput")
    mask_t = nc.dram_tensor("mask", [128, 1], F32, kind="ExternalInput")
    em_t = nc.dram_tensor("em_part", [S, 17], F32, kind="ExternalOutput")
    hc_t = nc.dram_tensor("hc_out", [128, 4], F32, kind="ExternalOutput")

    NBLK = S // 128
    with tile.TileContext(nc) as tc:
        from contextlib import ExitStack
        with ExitStack() as ctx:
            consts = ctx.enter_context(tc.tile_pool(name="consts", bufs=1))
            blk = ctx.enter_context(tc.tile_pool(name="blk", bufs=2))
            sp = ctx.enter_context(tc.tile_pool(name="steps", bufs=4))
            pp = ctx.enter_context(tc.tile_pool(name="psx", bufs=2, space="PSUM"))
            gp = ctx.enter_context(tc.tile_pool(name="gps", bufs=4, space="PSUM"))

            wih_sb = consts.tile([128, 8, 8, 128], F32)
            whh_sb = consts.tile([128, 2, 8, 128], F32)
            bias_sb = consts.tile([128, 8], F32)
            woT_sb = consts.tile([128, 2, 17], F32)
            mask_sb = consts.tile([128, 1], F32)
            ccarry = consts.tile([128, 2], F32)
            nc.sync.dma_start(wih_sb[:], _rview(wih_tt[:], [128, 8, 8, 128]))
            nc.sync.dma_start(whh_sb[:], _rview(whh_tt[:], [128, 2, 8, 128]))
            nc.sync.dma_start(bias_sb[:], bias_t[:])
            nc.sync.dma_start(woT_sb[:], _rview(woT_t[:], [128, 2, 17]))
            nc.sync.dma_start(mask_sb[:], mask_t[:])
            nc.gpsimd.memset(ccarry[:], 0.0)

            whh_r = consts.tile([128, 2, 8, 128], F32R)
            nc.vector.tensor_copy(whh_r[:], whh_sb[:])
            hpad = consts.tile([128, 2, 2], F32R)
            zz = consts.tile([128, 4], F32)
            nc.gpsimd.memset(zz[:], 0.0)
            nc.vector.tensor_copy(hpad[:], zz[:].rearrange("p (a b) -> p a b", a=2))

            xT_r = xT_t[:].rearrange("(ke p) s -> p ke s", p=128)

            with tc.For_i(0, NBLK, name="chain") as ib:
                xblk = blk.tile([128, 8, 128], F32, tag="xblk")
                nc.sync.dma_start(xblk[:], xT_r[:, :, bass.ds(ib * 128, 128)])

                xg = blk.tile([128, 8, 128], F32, tag="xg")
                for j in range(8):
                    psx = pp.tile([128, 128], F32, tag="psx")
                    for ke in range(8):
                        nc.tensor.matmul(psx[:], wih_sb[:, ke, j, :], xblk[:, ke, :],
                                         start=(ke == 0), stop=(ke == 7))
                    nc.scalar.activation(xg[:, j, :], psx[:], AF.Identity,
                                         bias=bias_sb[:, j:j + 1])

                hblk = blk.tile([128, 2, 130], F32R, tag="hblk")
                nc.vector.tensor_copy(hblk[:, :, 0:2], hpad[:])

                for t in range(128):
                    gps = gp.tile([128, 8, 2], F32, tag="gps")
                    for j in range(8):
                        for k in range(2):
                            nc.tensor.matmul(gps[:, j, :], whh_r[:, k, j, :],
                                             hblk[:, k, t:t + 2],
                                             start=(k == 0), stop=(k == 1))
                    g = sp.tile([128, 8], F32, tag="g")
                    nc.vector.tensor_add(g[:], gps[:, :, 1], xg[:, :, t])
                    s8 = sp.tile([128, 8], F32, tag="s8")
                    nc.scalar.activation(s8[:], g[:], AF.Sigmoid)
                    P = sp.tile([128, 2], F32, tag="P")
                    Q = sp.tile([128, 2], F32, tag="Q")
                    R = sp.tile([128, 2], F32, tag="R")
                    nc.vector.tensor_mul(P[:], s8[:, 0:2], s8[:, 6:8])
                    nc.vector.tensor_mul(Q[:], s8[:, 2:4], ccarry[:])
                    nc.vector.scalar_tensor_tensor(R[:], P[:], 2.0, s8[:, 0:2],
                                                   op0=mybir.AluOpType.mult,
                                                   op1=mybir.AluOpType.subtract)
                    nc.vector.tensor_add(ccarry[:], Q[:], R[:])
                    tct = sp.tile([128, 2], F32, tag="tc")
                    nc.scalar.activation(tct[:], ccarry[:], AF.Tanh)
                    nc.vector.tensor_mul(hblk[:, :, t + 2], s8[:, 4:6], tct[:])

                hblk_f = hblk[:].bitcast(F32)
                emps = pp.tile([128, 17], F32, tag="emps")
                for j in range(2):
                    nc.tensor.matmul(emps[:], hblk_f[:, j, 2:130], woT_sb[:, j, :],
                                     start=(j == 0), stop=(j == 1))
                em_sb = sp.tile([128, 17], F32, tag="em")
                nc.vector.tensor_scalar_mul(em_sb[:], emps[:], mask_sb[:])
                nc.sync.dma_start(em_t[:][bass.ds(ib * 128, 128), :], em_sb[:])

                nc.vector.tensor_copy(hpad[:], hblk[:, :, 128:130])

            hc = sp.tile([128, 4], F32, tag="hc")
            nc.vector.tensor_copy(hc[:, 0:2], hpad[:].bitcast(F32)[:, :, 1])
            nc.vector.tensor_copy(hc[:, 2:4], ccarry[:])
            nc.sync.dma_start(hc_t[:], hc[:])

    nc.compile()
    return nc


_NC_CACHE = None


def _get_program():
    global _NC_CACHE
    if _NC_CACHE is None:
        _NC_CACHE = _build_chain_program()
    return _NC_CACHE


def _logsumexp(a):
    m = a.max()
    return m + np.log(np.exp(a - m).sum())


def kernel(sentence, tags, w_ih_f, w_hh_f, b_ih_f, b_hh_f,
           w_ih_b, w_hh_b, b_ih_b, b_hh_b, w_out, b_out,
           start_trans, end_trans, trans):
    sentence = np.asarray(sentence, np.float32)
    tags = np.asarray(tags)
    f32 = lambda a: np.asarray(a, np.float32)
    w_ih_f, w_hh_f, b_ih_f, b_hh_f = map(f32, (w_ih_f, w_hh_f, b_ih_f, b_hh_f))
    w_ih_b, w_hh_b, b_ih_b, b_hh_b = map(f32, (w_ih_b, w_hh_b, b_ih_b, b_hh_b))
    w_out, b_out = f32(w_out), f32(b_out)
    start_trans, end_trans, trans = f32(start_trans), f32(end_trans), f32(trans)

    x = sentence[0]                                      # [S, E]
    nc = _get_program()

    wih_f, whh_f, bias_f, woT_f = _host_prep_dir(w_ih_f, w_hh_f, b_ih_f, b_hh_f, w_out, 0)
    wih_b, whh_b, bias_b, woT_b = _host_prep_dir(w_ih_b, w_hh_b, b_ih_b, b_hh_b, w_out, 256)

    xT_f = np.ascontiguousarray(x.T)
    xT_b = np.ascontiguousarray(x[::-1].T)
    ones_m = np.ones((128, 1), np.float32)
    zero_m = np.zeros((128, 1), np.float32)

    in_maps = []
    z_xT = np.zeros_like(xT_f)
    for c in range(8):
        if c == 0:
            m = dict(xT=xT_f, wih_t=wih_f, whh_t=whh_f, bias=bias_f, woT=woT_f,
                     mask=ones_m)
        elif c == 1:
            m = dict(xT=xT_b, wih_t=wih_b, whh_t=whh_b, bias=bias_b,
                     woT=woT_b, mask=ones_m)
        else:
            m = dict(xT=z_xT, wih_t=wih_f, whh_t=whh_f, bias=bias_f, woT=woT_f,
                     mask=zero_m)
        in_maps.append(m)

    res = run_bass_kernel_spmd(nc, in_maps, core_ids=list(range(8)))
    em_f = res.results[0]["em_part"]
    em_b = res.results[1]["em_part"][::-1]               # un-reverse
    em = em_f + em_b + b_out                              # [S, 17]

    tg = np.asarray(tags[0], np.int64)

    # --- gold score ---
    score = np.float32(start_trans[tg[0]]
                       + em[np.arange(S), tg].sum(dtype=np.float32)
                       + trans[tg[:-1], tg[1:]].sum(dtype=np.float32)
                       + end_trans[tg[-1]])

    # --- forward algorithm (logZ) and Viterbi, mirroring the reference ---
    alpha = start_trans + em[0]
    v = alpha.copy()
    bps = np.zeros((S - 1, T), np.int64)
    for t in range(1, S):
        a_s = alpha[:, None] + trans                      # [T, T]
        m = a_s.max(0)
        alpha = m + np.log(np.exp(a_s - m).sum(0)) + em[t]
        s_v = v[:, None] + trans
        bps[t - 1] = s_v.argmax(0)
        v = s_v.max(0) + em[t]

    logZ = _logsumexp(alpha + end_trans)
    loss = -(score - np.float32(logZ)) * np.float32(10.0)

    last = int(np.argmax(v + end_trans))
    path = np.zeros(S, np.int64)
    path[S - 1] = last
    cur = last
    for t in range(S - 2, -1, -1):
        cur = bps[t][cur]
        path[t] = cur

    path = path.astype(np.int32)
    return np.float32(loss), path


# revision 3
# speedup vs baseline: 1.0678x; 1.0678x over previous
"""BiLSTM-CRF loss kernel for Trainium2 (8 NeuronCores, SPMD).

Device: the two 8192-step LSTM chains (forward on core 0, backward on core 1,
same SPMD program, data-differentiated; cores 2-7 run the program on zeros and
contribute masked-out zero partials). Each chain core JIT-computes its input
projection (x @ w_ih.T) block-by-block, runs the recurrence with gates laid
out [128 partitions x 8 gate-columns], and emits its half of the emission
matrix (h_dir @ w_out_dir.T) per 128-step block.

Host: final CRF dynamic programs (forward logsumexp, Viterbi, gold score) are
sequential 17-wide recursions identical to the reference implementation.
"""
import sys
import numpy as np

sys.path.insert(0, "/opt/trn_rl_repo")

import concourse.bass as bass
import concourse.bacc as bacc
import concourse.tile as tile
import concourse.mybir as mybir
from concourse.bass_utils import run_bass_kernel_spmd

F32 = mybir.dt.float32
F32R = mybir.dt.float32r
AF = mybir.ActivationFunctionType

S, E, H, T = 8192, 1024, 512, 17
H2 = H // 2

GATE_PERM = np.concatenate([
    np.arange(0, 256),      # i
    np.arange(256, 512),    # f
    np.arange(768, 1024),   # o
    np.arange(512, 768),    # g~ (tanh gate, pre-scaled by 2 for sigmoid trick)
])


def _host_prep_dir(w_ih, w_hh, b_ih, b_hh, w_out, off):
    wih_perm = w_ih[GATE_PERM].copy()
    whh_perm = w_hh[GATE_PERM].copy()
    b_comb = (b_ih + b_hh)[GATE_PERM].copy()
    # tanh(z) = 2*sigmoid(2z) - 1: fold the 2x into the g~ gate rows so one
    # sigmoid activation covers all 8 gate columns.
    wih_perm[768:1024] *= 2.0
    whh_perm[768:1024] *= 2.0
    b_comb[768:1024] *= 2.0
    B = wih_perm.reshape(8, 128, 8, 128)                 # [j, m, ke, p]
    wih_t = np.ascontiguousarray(B.transpose(3, 2, 0, 1)).reshape(128, 8 * 8 * 128)
    A = whh_perm.reshape(8, 128, 2, 128)                 # [j, m, k, p]
    whh_t = np.ascontiguousarray(A.transpose(3, 2, 0, 1)).reshape(128, 2 * 8 * 128)
    bias = np.ascontiguousarray(b_comb.reshape(8, 128).T)
    C = w_out[:, off:off + 256].T.reshape(2, 128, 17)    # [j, p, tau]
    woT = np.ascontiguousarray(C.transpose(1, 0, 2)).reshape(128, 34)
    return (wih_t.astype(np.float32), whh_t.astype(np.float32),
            bias.astype(np.float32), woT.astype(np.float32))


def _rview(ap, shape):
    names = "abcdefg"
    src = f"{names[0]} ({' '.join(names[1:len(shape)])})"
    dst = ' '.join(names[:len(shape)])
    kw = {names[i]: shape[i] for i in range(1, len(shape))}
    return ap.rearrange(f"{src} -> {dst}", **kw)


def _build_chain_program():
    nc = bacc.Bacc("TRN2", target_bir_lowering=False, debug=False, num_devices=8)
    xT_t = nc.dram_tensor("xT", [1024, S], F32, kind="ExternalInput")
    wih_tt = nc.dram_tensor("wih_t", [128, 8 * 8 * 128], F32, kind="ExternalInput")
    whh_tt = nc.dram_tensor("whh_t", [128, 2 * 8 * 128], F32, kind="ExternalInput")
    bias_t = nc.dram_tensor("bias", [128, 8], F32, kind="ExternalInput")
    woT_t = nc.dram_tensor("woT", [128, 34], F32, kind="ExternalInput")
    mask_t = nc.dram_tensor("mask", [128, 1], F32, kind="ExternalInput")
    em_t = nc.dram_tensor("em_part", [S, 17], F32, kind="ExternalOutput")
    hc_t = nc.dram_tensor("hc_out", [128, 4], F32, kind="ExternalOutput")

    NBLK = S // 128
    with tile.TileContext(nc) as tc:
        from contextlib import ExitStack
        with ExitStack() as ctx:
            consts = ctx.enter_context(tc.tile_pool(name="consts", bufs=1))
            blk = ctx.enter_context(tc.tile_pool(name="blk", bufs=2))
            sp = ctx.enter_context(tc.tile_pool(name="steps", bufs=4))
            pp = ctx.enter_context(tc.tile_pool(name="psx", bufs=2, space="PSUM"))
            gp = ctx.enter_context(tc.tile_pool(name="gps", bufs=4, space="PSUM"))

            wih_sb = consts.tile([128, 8, 8, 128], F32)
            whh_sb = consts.tile([128, 2, 8, 128], F32)
            bias_sb = consts.tile([128, 8], F32)
            woT_sb = consts.tile([128, 2, 17], F32)
            mask_sb = consts.tile([128, 1], F32)
            ccarry = consts.tile([128, 2], F32)
            nc.sync.dma_start(wih_sb[:], _rview(wih_tt[:], [128, 8, 8, 128]))
            nc.sync.dma_start(whh_sb[:], _rview(whh_tt[:], [128, 2, 8, 128]))
            nc.sync.dma_start(bias_sb[:], bias_t[:])
            nc.sync.dma_start(woT_sb[:], _rview(woT_t[:], [128, 2, 17]))
            nc.sync.dma_start(mask_sb[:], mask_t[:])
            nc.gpsimd.memset(ccarry[:], 0.0)

            whh_r = consts.tile([128, 2, 8, 128], F32R)
            nc.vector.tensor_copy(whh_r[:], whh_sb[:])
            hpad = consts.tile([128, 2, 2], F32R)
            zz = consts.tile([128, 4], F32)
            nc.gpsimd.memset(zz[:], 0.0)
            nc.vector.tensor_copy(hpad[:], zz[:].rearrange("p (a b) -> p a b", a=2))

            xT_r = xT_t[:].rearrange("(ke p) s -> p ke s", p=128)

            with tc.For_i(0, NBLK, name="chain") as ib:
                xblk = blk.tile([128, 8, 128], F32, tag="xblk")
                nc.sync.dma_start(xblk[:], xT_r[:, :, bass.ds(ib * 128, 128)])

                xg = blk.tile([128, 8, 128], F32, tag="xg")
                for j in range(8):
                    psx = pp.tile([128, 128], F32, tag="psx")
                    for ke in range(8):
                        nc.tensor.matmul(psx[:], wih_sb[:, ke, j, :], xblk[:, ke, :],
                                         start=(ke == 0), stop=(ke == 7))
                    nc.scalar.activation(xg[:, j, :], psx[:], AF.Identity,
                                         bias=bias_sb[:, j:j + 1])

                hblk = blk.tile([128, 2, 130], F32R, tag="hblk")
                nc.vector.tensor_copy(hblk[:, :, 0:2], hpad[:])

                for t in range(128):
                    gps = gp.tile([128, 8, 2], F32, tag="gps")
                    for j in range(8):
                        for k in range(2):
                            nc.tensor.matmul(gps[:, j, :], whh_r[:, k, j, :],
                                             hblk[:, k, t:t + 2],
                                             start=(k == 0), stop=(k == 1))
                    g = sp.tile([128, 8], F32, tag="g")
                    nc.vector.tensor_add(g[:], gps[:, :, 1], xg[:, :, t])
                    s8 = sp.tile([128, 8], F32, tag="s8")
                    nc.scalar.activation(s8[:], g[:], AF.Sigmoid)
                    P = sp.tile([128, 2], F32, tag="P")
                    Q = sp.tile([128, 2], F32, tag="Q")
                    R = sp.tile([128, 2], F32, tag="R")
                    nc.vector.tensor_mul(P[:], s8[:, 0:2], s8[:, 6:8])
                    nc.vector.tensor_mul(Q[:], s8[:, 2:4], ccarry[:])
                    nc.vector.scalar_tensor_tensor(R[:], P[:], 2.0, s8[:, 0:2],
                                                   op0=mybir.AluOpType.mult,
                                                   op1=mybir.AluOpType.subtract)
                    nc.vector.tensor_add(ccarry[:], Q[:], R[:])
                    tct = sp.tile([128, 2], F32, tag="tc")
                    nc.scalar.activation(tct[:], ccarry[:], AF.Tanh)
                    nc.vector.tensor_mul(hblk[:, :, t + 2], s8[:, 4:6], tct[:])

                hblk_f = hblk[:].bitcast(F32)
                emps = pp.tile([128, 17], F32, tag="emps")
                for j in range(2):
                    nc.tensor.matmul(emps[:], hblk_f[:, j, 2:130], woT_sb[:, j, :],
                                     start=(j == 0), stop=(j == 1))
                em_sb = sp.tile([128, 17], F32, tag="em")
                nc.vector.tensor_scalar_mul(em_sb[:], emps[:], mask_sb[:])
                nc.sync.dma_start(em_t[:][bass.ds(ib * 128, 128), :], em_sb[:])

                nc.vector.tensor_copy(hpad[:], hblk[:, :, 128:130])

            hc = sp.tile([128, 4], F32, tag="hc")
            nc.vector.tensor_copy(hc[:, 0:2], hpad[:].bitcast(F32)[:, :, 1])
            nc.vector.tensor_copy(hc[:, 2:4], ccarry[:])
            nc.sync.dma_start(hc_t[:], hc[:])

    nc.compile()
    return nc


_NC_CACHE = None
LAST_DEVICE_SECONDS = 0.0


def _get_program():
    global _NC_CACHE
    if _NC_CACHE is None:
        _NC_CACHE = _build_chain_program()
    return _NC_CACHE


def _logsumexp(a):
    m = a.max()
    return m + np.log(np.exp(a - m).sum())


def kernel(sentence, tags, w_ih_f, w_hh_f, b_ih_f, b_hh_f,
           w_ih_b, w_hh_b, b_ih_b, b_hh_b, w_out, b_out,
           start_trans, end_trans, trans):
    sentence = np.asarray(sentence, np.float32)
    tags = np.asarray(tags)
    f32 = lambda a: np.asarray(a, np.float32)
    w_ih_f, w_hh_f, b_ih_f, b_hh_f = map(f32, (w_ih_f, w_hh_f, b_ih_f, b_hh_f))
    w_ih_b, w_hh_b, b_ih_b, b_hh_b = map(f32, (w_ih_b, w_hh_b, b_ih_b, b_hh_b))
    w_out, b_out = f32(w_out), f32(b_out)
    start_trans, end_trans, trans = f32(start_trans), f32(end_trans), f32(trans)

    x = sentence[0]                                      # [S, E]
    nc = _get_program()

    wih_f, whh_f, bias_f, woT_f = _host_prep_dir(w_ih_f, w_hh_f, b_ih_f, b_hh_f, w_out, 0)
    wih_b, whh_b, bias_b, woT_b = _host_prep_dir(w_ih_b, w_hh_b, b_ih_b, b_hh_b, w_out, 256)

    xT_f = np.ascontiguousarray(x.T)
    xT_b = np.ascontiguousarray(x[::-1].T)
    ones_m = np.ones((128, 1), np.float32)
    zero_m = np.zeros((128, 1), np.float32)

    in_maps = []
    z_xT = np.zeros_like(xT_f)
    for c in range(8):
        if c == 0:
            m = dict(xT=xT_f, wih_t=wih_f, whh_t=whh_f, bias=bias_f, woT=woT_f,
                     mask=ones_m)
        elif c == 1:
            m = dict(xT=xT_b, wih_t=wih_b, whh_t=whh_b, bias=bias_b,
                     woT=woT_b, mask=ones_m)
        else:
            m = dict(xT=z_xT, wih_t=wih_f, whh_t=whh_f, bias=bias_f, woT=woT_f,
                     mask=zero_m)
        in_maps.append(m)

    import time as _time
    global LAST_DEVICE_SECONDS
    _t0 = _time.time()
    try:
        res = run_bass_kernel_spmd(nc, in_maps, core_ids=list(range(8)))
    except Exception:
        # transient axon/PJRT failures happen; retry once
        res = run_bass_kernel_spmd(nc, in_maps, core_ids=list(range(8)))
    LAST_DEVICE_SECONDS = _time.time() - _t0
    em_f = res.results[0]["em_part"]
    em_b = res.results[1]["em_part"][::-1]               # un-reverse
    em = em_f + em_b + b_out                              # [S, 17]

    tg = np.asarray(tags[0], np.int64)

    # --- gold score ---
    score = np.float32(start_trans[tg[0]]
                       + em[np.arange(S), tg].sum(dtype=np.float32)
                       + trans[tg[:-1], tg[1:]].sum(dtype=np.float32)
                       + end_trans[tg[-1]])

    # --- forward algorithm (logZ) and Viterbi, mirroring the reference ---
    alpha = start_trans + em[0]
    v = alpha.copy()
    bps = np.zeros((S - 1, T), np.int64)
    for t in range(1, S):
        a_s = alpha[:, None] + trans                      # [T, T]
        m = a_s.max(0)
        alpha = m + np.log(np.exp(a_s - m).sum(0)) + em[t]
        s_v = v[:, None] + trans
        bps[t - 1] = s_v.argmax(0)
        v = s_v.max(0) + em[t]

    logZ = _logsumexp(alpha + end_trans)
    loss = -(score - np.float32(logZ)) * np.float32(10.0)

    last = int(np.argmax(v + end_trans))
    path = np.zeros(S, np.int64)
    path[S - 1] = last
    cur = last
    for t in range(S - 2, -1, -1):
        cur = bps[t][cur]
        path[t] = cur

    path = path.astype(np.int32)
    return np.float32(loss), path
